# revision 1
# baseline (speedup 1.0000x reference)
"""BitNet MLP (act_quant -> ternary matmul -> relu^2 -> SubLN -> act_quant ->
ternary matmul) on 8 Trainium2 NeuronCores, data-parallel over tokens.

Math notes (exactness):
- act_quant int levels (|q| <= 127) and ternary weights {-1,0,1} are exactly
  representable in bf16, so both matmuls run on the PE in bf16 with exact
  integer arithmetic (f32 PSUM accumulation, |sums| < 2^24).
- All quantization scales are folded into per-token scalars applied to the
  final [tok, 512] output: out = i2 * beta_t with
    beta_t = clip(c_t * alpha_t * Sabs_t, 1e-5) * clip(mean|w_dn|,1e-5) / 127
  where alpha_t = (clip(max|x_t|,1e-5) * clip(mean|w_up|,1e-5) / 127)^2,
  Sabs_t = max_i |relu(ih)^2 * g|, c_t = rsqrt(var_t + 1e-6).
- Rounding uses the magic-number trick (x + 1.5*2^23 - 1.5*2^23) == RNE
  round-to-integer for |x| < 2^22, matching jnp.round (half-to-even).
- SubLN variance is recovered from the quantized intermediate:
  var = alpha^2 * sum(iu^2) * (Sabs/127)^2 / (2048 * g0^2); the
  quantization error on sum(iu^2) is ~0.1% which is far below tolerance.
  (For non-constant g an extra pass computes sum((relu^2)^2) directly.)
"""
import os
import numpy as np

import concourse.bass as bass
import concourse.tile as tile
from concourse import mybir
from concourse.bass_utils import run_bass_kernel_spmd
from concourse.masks import make_identity

# ---------------------------------------------------------------------------
# Workaround for walrus "Too many sync wait commands" on the TileContext tail
# drain: split the drain's semaphore waits across single-wait SP NOPs, then
# advance the observed clocks so the real drain needs none.
import re as _re
import bass_rust as _bass_rust


def _patched_drain_and_barrier(self, tick_clock, wait_clock):
    gc = tick_clock.global_clock
    ticks = list(map(int, _re.findall(r"\d+", repr(gc))))
    n = len(ticks)
    nonzero = [(i, t) for i, t in enumerate(ticks) if t > 0]
    for i, t in nonzero:
        sub = [0] * n
        sub[i] = t
        sub_scoped = _bass_rust.ScopedClock({None: _bass_rust.VectorClock(sub)})
        nop = self.nc.sync.nop()
        wait_clock.add_sem_waits(nop.ins, sub_scoped)
        for ec in wait_clock.engine_clocks:
            ec.update_past(sub_scoped)
    drain_inst = self.nc.sync.drain()
    wait_clock.add_sem_waits(drain_inst.ins,
                             _bass_rust.ScopedClock({None: gc}))
    self.nc.all_engine_barrier()
    popped = self.nc._tile_sem_poison_stack.pop()
    assert popped is self._sem_poison
    self.nc.clear_and_free_semaphores(list(self.sems.allocated().values()))
    self.nc.all_engine_barrier()


tile.TileContext._drain_and_barrier = _patched_drain_and_barrier


def _split_sync_waits(nc, keep_default=1):
    """walrus caps the number of semaphore waits a single instruction can
    carry (CTRL ops take only 1; compute ops a few). Hoist excess waits onto
    single-wait NOPs inserted immediately before the instruction on the same
    engine — identical semantics, engines execute in order."""
    import dataclasses
    keep_by_op = {}
    proto = None
    for f in nc.m.functions:
        for bb in f.blocks:
            for inst in bb.instructions:
                if type(inst).__name__ == "InstNoOp":
                    proto = inst
                    break
            if proto is not None:
                break
        if proto is not None:
            break
    counter = [0]
    for f in nc.m.functions:
        new_blocks = []
        for bb in f.blocks:
            out = []
            changed = False
            for inst in bb.instructions:
                si = inst.sync_info
                ow = list(si.on_wait) if si is not None and si.on_wait else []
                keep = keep_by_op.get(inst.opcode, keep_default)
                if len(ow) > keep:
                    assert proto is not None, "no NoOp prototype found yet"
                    for w in ow[:-keep]:
                        counter[0] += 1
                        nop = dataclasses.replace(
                            proto,
                            name=f"I-waitsplit-{counter[0]}",
                            engine=inst.engine,
                            sync_info=_bass_rust.SyncInfo(on_wait=[w],
                                                          on_update=[]),
                        )
                        out.append(nop)
                    si.on_wait = ow[-keep:]
                    changed = True
                out.append(inst)
            if changed:
                bb2 = _bass_rust.BasicBlock(name=bb.name, instructions=out)
                bb2.IsExit = bb.IsExit
                bb2.IsLoopEntry = bb.IsLoopEntry
                bb2.IsPredicated = bb.IsPredicated
                new_blocks.append(bb2)
            else:
                new_blocks.append(bb)
        f.blocks = new_blocks
# ---------------------------------------------------------------------------

F32 = mybir.dt.float32
BF16 = mybir.dt.bfloat16
ALU = mybir.AluOpType
AF = mybir.ActivationFunctionType

N_CORES = 8
B, S, H, I = 8, 8192, 512, 2048
TOK = B * S                  # 65536 tokens total
TPC = TOK // N_CORES         # 8192 tokens per core
P = 128                      # partition tile
NT = TPC // P                # 64 token tiles per core
NKH = H // P                 # 4 k-tiles over H
NKI = I // P                 # 16 k-tiles over I
NB = I // 512                # 4 psum banks for the up matmul

MAGIC = 12582912.0           # 1.5 * 2^23: RNE round-to-int trick
EPS = 1e-6                   # SubLN eps (from reference)

LAST_RESULT = None           # set by kernel() for test harness introspection


def _emit_weight_quant(nc, stage, junkp, ps, consts, wT_dram, n_ktiles,
                       nsub, name, magicb):
    """Quantize a (host-pre-transposed) weight matrix to ternary bf16 tiles.

    wT_dram: [n_ktiles*128, nsub*512] f32 in DRAM (contraction dim on rows).
    Returns (list of [128, nsub, 512] bf16 sbuf tiles, scale_recip [128,1],
    meanclip [128,1]) where meanclip = clip(mean|w|, 1e-5) broadcast to all
    partitions.
    """
    n_elem = n_ktiles * 128 * nsub * 512

    # pass 1: per-partition abs sums
    asum = consts.tile([P, n_ktiles], F32, tag=f"{name}_asum")
    for k in range(n_ktiles):
        wf = stage.tile([P, nsub * 512], F32, tag="stage")
        nc.gpsimd.dma_start(out=wf, in_=wT_dram[k * P:(k + 1) * P, :])
        junk = junkp.tile([P, nsub * 512], BF16, tag="junk")
        nc.scalar.activation(out=junk, in_=wf, func=AF.Abs,
                             accum_out=asum[:, k:k + 1])
    tot = consts.tile([P, 1], F32, tag=f"{name}_tot")
    nc.vector.tensor_reduce(out=tot, in_=asum, axis=mybir.AxisListType.X,
                            op=ALU.add)
    # broadcast-sum across partitions: ones128.T @ tot
    ones128 = stage.tile([P, P], F32, tag="ones128")
    nc.vector.memset(ones128, 1.0)
    totp = ps.tile([P, 1], F32, tag="totp")
    nc.tensor.matmul(out=totp, lhsT=ones128, rhs=tot, start=True, stop=True)
    gsum = consts.tile([P, 1], F32, tag=f"{name}_gsum")
    nc.scalar.copy(out=gsum, in_=totp)
    # mean -> clip -> reciprocal scale
    meanclip = consts.tile([P, 1], F32, tag=f"{name}_meanclip")
    nc.vector.tensor_scalar(out=meanclip, in0=gsum, scalar1=1.0 / n_elem,
                            scalar2=1e-5, op0=ALU.mult, op1=ALU.max)
    swq = consts.tile([P, 1], F32, tag=f"{name}_swq")
    nc.vector.reciprocal(out=swq, in_=meanclip)

    # pass 2: re-load, round+clip to ternary bf16
    wq_tiles = []
    for k in range(n_ktiles):
        wf = stage.tile([P, nsub * 512], F32, tag="stage")
        nc.gpsimd.dma_start(out=wf, in_=wT_dram[k * P:(k + 1) * P, :])
        rt = stage.tile([P, nsub * 512], F32, tag="stage_rt")
        nc.scalar.activation(out=rt, in_=wf, func=AF.Identity,
                             bias=magicb, scale=swq)
        cl = stage.tile([P, nsub * 512], F32, tag="stage_cl")
        nc.vector.tensor_scalar(out=cl, in0=rt, scalar1=MAGIC, scalar2=1.0,
                                op0=ALU.subtract, op1=ALU.min)
        wq = consts.tile([P, nsub, 512], BF16, tag=f"{name}_wq{k}")
        nc.vector.tensor_scalar(out=wq.rearrange("p a b -> p (a b)"), in0=cl,
                                scalar1=-1.0, scalar2=None, op0=ALU.max)
        wq_tiles.append(wq)
    return wq_tiles, meanclip


def build_nc(general_g: bool):
    nc = bass.Bass()
    x_d = nc.dram_tensor("x", [TPC, H], F32, kind="ExternalInput")
    wupT_d = nc.dram_tensor("wupT", [H, I], F32, kind="ExternalInput")
    wdnT_d = nc.dram_tensor("wdnT", [I, H], F32, kind="ExternalInput")
    g_d = nc.dram_tensor("g", [I], F32, kind="ExternalInput")
    out_d = nc.dram_tensor("out", [TPC, H], F32, kind="ExternalOutput")

    from contextlib import ExitStack
    with ExitStack() as ctx:
        tc = ctx.enter_context(tile.TileContext(nc))

        # ---------------- constants / weight prep ----------------
        consts = ctx.enter_context(tc.tile_pool(name="consts", bufs=1))

        ident = consts.tile([P, P], BF16)
        make_identity(nc, ident)

        magicb = consts.tile([P, 1], F32)
        nc.vector.memset(magicb, MAGIC)

        # g broadcast to all partitions: [128, I] f32
        g_bc = consts.tile([P, I], F32)
        g_ap = g_d[:]
        g_bcast_ap = bass.AP(tensor=g_ap.tensor, offset=g_ap.offset,
                             ap=[[0, P]] + list(g_ap.ap))
        nc.gpsimd.dma_start(out=g_bc, in_=g_bcast_ap)

        g0b = consts.tile([P, 1], F32)
        with tc.tile_pool(name="wstage", bufs=2) as stage, \
                tc.tile_pool(name="wjunk", bufs=2) as junkp, \
                tc.tile_pool(name="wps", bufs=1, space="PSUM") as wps:
            # g0 broadcast [128,1] via K=1 matmul with ones
            ones_row = stage.tile([1, P], F32, tag="ones_row")
            nc.vector.memset(ones_row, 1.0)
            g0_sb = stage.tile([1, 1], F32, tag="g0sb")
            nc.gpsimd.dma_start(out=g0_sb, in_=g_d[0:1])
            g0_ps = wps.tile([P, 1], F32, tag="g0ps")
            nc.tensor.matmul(out=g0_ps, lhsT=ones_row, rhs=g0_sb, start=True,
                             stop=True)
            nc.scalar.copy(out=g0b, in_=g0_ps)

            wup_q, up_meanclip = _emit_weight_quant(
                nc, stage, junkp, wps, consts, wupT_d, NKH, NB, "wup", magicb)
            wdn_q, dn_meanclip = _emit_weight_quant(
                nc, stage, junkp, wps, consts, wdnT_d, NKI, 1, "wdn", magicb)

        # k1b = clip(mean|w_up|,1e-5)/127  (per-token gamma multiplier)
        k1b = consts.tile([P, 1], F32)
        nc.vector.tensor_scalar_mul(out=k1b, in0=up_meanclip, scalar1=1.0 / 127.0)
        # wdk = clip(mean|w_dn|,1e-5)/127  (final output multiplier)
        wdk = consts.tile([P, 1], F32)
        nc.vector.tensor_scalar_mul(out=wdk, in0=dn_meanclip, scalar1=1.0 / 127.0)
        # sg127 = sign(g0)*127 (quant scale sign), g0a = |g0|
        sg127 = consts.tile([P, 1], F32)
        nc.scalar.activation(out=sg127, in_=g0b, func=AF.Sign)
        nc.vector.tensor_scalar_mul(out=sg127, in0=sg127, scalar1=127.0)
        g0a = consts.tile([P, 1], F32)
        nc.scalar.activation(out=g0a, in_=g0b, func=AF.Abs)

        # ---------------- main token-tile pipeline ----------------
        # isg = sign(g0)/127 (or 1/127 for general g): folds the quant scale
        # sign so d = recip(clip(S)*isg) = sign*127/clip(S) in 2 small ops.
        isg = consts.tile([P, 1], F32)
        if general_g:
            nc.vector.memset(isg, 1.0 / 127.0)
        else:
            nc.vector.tensor_scalar_mul(out=isg, in0=sg127,
                                        scalar1=1.0 / (127.0 * 127.0))

        BG = 8  # tiles per small-op batch
        KV = 1.0 / (127.0 * 127.0 * I)

        xs_pool = ctx.enter_context(tc.tile_pool(name="xs", bufs=2 * BG))
        xq_pool = ctx.enter_context(tc.tile_pool(name="xqp", bufs=3))
        big = ctx.enter_context(tc.tile_pool(name="big", bufs=2))
        iup = ctx.enter_context(tc.tile_pool(name="iup", bufs=3))
        outp = ctx.enter_context(tc.tile_pool(name="outp", bufs=BG + 1))
        o2p = ctx.enter_context(tc.tile_pool(name="o2p", bufs=3))
        junkp = ctx.enter_context(tc.tile_pool(name="mjunk", bufs=1))
        small = ctx.enter_context(tc.tile_pool(name="small", bufs=3))
        batchp = ctx.enter_context(tc.tile_pool(name="batchp", bufs=2))
        ps_xT = ctx.enter_context(tc.tile_pool(name="ps_xT", bufs=1,
                                               space="PSUM"))
        ps_ih = ctx.enter_context(tc.tile_pool(name="ps_ih", bufs=1,
                                               space="PSUM"))
        ps_iuT = ctx.enter_context(tc.tile_pool(name="ps_iuT", bufs=1,
                                                space="PSUM"))
        ps_o = ctx.enter_context(tc.tile_pool(name="ps_o", bufs=1,
                                              space="PSUM"))

        IH2 = I // 2  # up-matmul accumulates in two 2-bank halves

        def phase_a(ib, state):
            """DMA x tiles + per-token absmax, then batched x-scale chain."""
            xm8 = batchp.tile([P, BG], F32, tag="xm8")
            x_tiles = []
            for j in range(BG):
                r0 = (ib + j) * P
                x_sb = xs_pool.tile([P, H], F32, tag="x")
                nc.sync.dma_start(out=x_sb, in_=x_d[r0:r0 + P, :])
                x_tiles.append(x_sb)
                nc.vector.tensor_reduce(out=xm8[:, j:j + 1], in_=x_sb,
                                        axis=mybir.AxisListType.X, op=ALU.max,
                                        apply_absolute_value=True)
            t08 = batchp.tile([P, BG], F32, tag="t08")
            nc.vector.tensor_scalar_max(out=t08, in0=xm8, scalar1=1e-5)
            xr8 = batchp.tile([P, BG], F32, tag="xr8")
            nc.vector.reciprocal(out=xr8, in_=t08)
            xsc8 = batchp.tile([P, BG], F32, tag="xsc8")
            nc.vector.tensor_scalar_mul(out=xsc8, in0=xr8, scalar1=127.0)
            state[ib] = (x_tiles, t08, xsc8)

        def phase_bc(ib, state):
            x_tiles, t08, xsc8 = state.pop(ib)
            Sm8 = batchp.tile([P, BG], F32, tag="Sm8")
            q28 = batchp.tile([P, BG], F32, tag="q28")
            q2g8 = None
            if general_g:
                q2g8 = batchp.tile([P, BG], F32, tag="q2g8")
            o_tiles = []

            for j in range(BG):
                x_sb = x_tiles[j]
                # quantize x (RNE round via magic): ACT + DVE
                xq = xq_pool.tile([P, H], F32, tag="xq")
                nc.scalar.activation(out=xq, in_=x_sb, func=AF.Identity,
                                     bias=magicb, scale=xsc8[:, j:j + 1])
                ix = xq_pool.tile([P, H], BF16, tag="ix")
                nc.vector.tensor_scalar(out=ix, in0=xq, scalar1=MAGIC,
                                        scalar2=None, op0=ALU.subtract)
                # transpose ix via PE, drain on ACT
                xT_ps = ps_xT.tile([P, NKH, P], BF16, tag="xT")
                for k in range(NKH):
                    nc.tensor.transpose(out=xT_ps[:, k, :],
                                        in_=ix[:, k * P:(k + 1) * P],
                                        identity=ident)
                xT_sb = xq_pool.tile([P, NKH, P], BF16, tag="xTsb")
                nc.scalar.copy(out=xT_sb, in_=xT_ps)

                # up matmul in two halves (each 2 PSUM banks) so the next
                # tile's matmuls only wait on a half-drain
                r_sb = big.tile([P, I], F32, tag="r")
                smh = small.tile([P, 2], F32, tag="smh")
                for h in range(2):
                    ihh = ps_ih.tile([P, IH2], F32, tag="ih")
                    for nb in range(2):
                        lo = nb * 512
                        for k in range(NKH):
                            nc.tensor.matmul(
                                out=ihh[:, lo:lo + 512],
                                lhsT=xT_sb[:, k, :],
                                rhs=wup_q[k][:, 2 * h + nb, :],
                                start=(k == 0), stop=(k == NKH - 1))
                    nc.scalar.activation(out=r_sb[:, h * IH2:(h + 1) * IH2],
                                         in_=ihh, func=AF.Relu)
                    if not general_g:
                        nc.vector.tensor_reduce(out=smh[:, h:h + 1], in_=ihh,
                                                axis=mybir.AxisListType.X,
                                                op=ALU.max)

                if general_g:
                    s_sb = big.tile([P, I], F32, tag="s")
                    nc.gpsimd.tensor_tensor(out=s_sb, in0=r_sb, in1=r_sb,
                                            op=ALU.mult)
                    sq_in = big.tile([P, I], F32, tag="sg")
                    nc.vector.tensor_tensor(out=sq_in, in0=s_sb, in1=g_bc,
                                            op=ALU.mult)
                    junk3 = junkp.tile([P, I], BF16, tag="junk3")
                    nc.scalar.activation(out=junk3, in_=s_sb, func=AF.Square,
                                         accum_out=q2g8[:, j:j + 1])
                    nc.vector.tensor_reduce(out=Sm8[:, j:j + 1], in_=sq_in,
                                            axis=mybir.AxisListType.X,
                                            op=ALU.max,
                                            apply_absolute_value=True)
                    sc2 = small.tile([P, 1], F32, tag="sc2")
                    nc.vector.tensor_scalar(out=sc2, in0=Sm8[:, j:j + 1],
                                            scalar1=1e-30, scalar2=isg,
                                            op0=ALU.max, op1=ALU.mult)
                    dr = small.tile([P, 1], F32, tag="dr")
                    nc.vector.reciprocal(out=dr, in_=sc2)
                    rt = big.tile([P, I], F32, tag="rt")
                    nc.vector.tensor_scalar(out=rt, in0=sq_in, scalar1=dr,
                                            scalar2=MAGIC, op0=ALU.mult,
                                            op1=ALU.add)
                    iu = iup.tile([P, I], BF16, tag="iu")
                    nc.vector.tensor_scalar(out=iu, in0=rt, scalar1=MAGIC,
                                            scalar2=None, op0=ALU.subtract)
                else:
                    # s' = r*r on GPSIMD
                    s_sb = big.tile([P, I], F32, tag="s")
                    nc.gpsimd.tensor_tensor(out=s_sb, in0=r_sb, in1=r_sb,
                                            op=ALU.mult)
                    # rmax = max over both halves (straight from PSUM above)
                    nc.vector.tensor_reduce(out=Sm8[:, j:j + 1], in_=smh,
                                            axis=mybir.AxisListType.X,
                                            op=ALU.max)
                    mr = small.tile([P, 1], F32, tag="mr")
                    nc.vector.tensor_scalar_max(out=mr, in0=Sm8[:, j:j + 1],
                                                scalar1=1e-15)
                    sc2 = small.tile([P, 1], F32, tag="sc2")
                    nc.vector.tensor_scalar(out=sc2, in0=mr, scalar1=mr,
                                            scalar2=isg, op0=ALU.mult,
                                            op1=ALU.mult)
                    dr = small.tile([P, 1], F32, tag="dr")
                    nc.vector.reciprocal(out=dr, in_=sc2)
                    rt = big.tile([P, I], F32, tag="rt")
                    nc.vector.tensor_scalar(out=rt, in0=s_sb, scalar1=dr,
                                            scalar2=MAGIC, op0=ALU.mult,
                                            op1=ALU.add)
                    iu = iup.tile([P, I], BF16, tag="iu")
                    nc.vector.tensor_scalar(out=iu, in0=rt, scalar1=MAGIC,
                                            scalar2=None, op0=ALU.subtract)

                # q2 = sum(iu^2) (ACT square + accumulate)
                junk2 = junkp.tile([P, I], BF16, tag="junk2")
                nc.scalar.activation(out=junk2, in_=iu, func=AF.Square,
                                     accum_out=q28[:, j:j + 1])

                # transpose iu via PE, drain on ACT
                iuT_ps = ps_iuT.tile([P, NKI, P], BF16, tag="iuT")
                for k in range(NKI):
                    nc.tensor.transpose(out=iuT_ps[:, k, :],
                                        in_=iu[:, k * P:(k + 1) * P],
                                        identity=ident)
                iuT_sb = iup.tile([P, NKI, P], BF16, tag="iuTsb")
                nc.scalar.copy(out=iuT_sb, in_=iuT_ps)

                # down matmul + plain drain (beta applied later, batched)
                o_ps = ps_o.tile([P, H], F32, tag="o")
                for k in range(NKI):
                    nc.tensor.matmul(out=o_ps, lhsT=iuT_sb[:, k, :],
                                     rhs=wdn_q[k][:, 0, :],
                                     start=(k == 0), stop=(k == NKI - 1))
                o_sb = outp.tile([P, H], F32, tag="osb")
                nc.scalar.copy(out=o_sb, in_=o_ps)
                o_tiles.append(o_sb)

            # --- batched beta chain ---
            scc8 = batchp.tile([P, BG], F32, tag="scc8")
            if general_g:
                nc.vector.tensor_scalar_max(out=scc8, in0=Sm8, scalar1=1e-30)
            else:
                ra8 = batchp.tile([P, BG], F32, tag="ra8")
                nc.vector.tensor_scalar_max(out=ra8, in0=Sm8, scalar1=0.0)
                ssq8 = batchp.tile([P, BG], F32, tag="ssq8")
                nc.vector.tensor_tensor(out=ssq8, in0=ra8, in1=ra8,
                                        op=ALU.mult)
                nc.vector.tensor_scalar_max(out=scc8, in0=ssq8,
                                            scalar1=1e-30)
            ga8 = batchp.tile([P, BG], F32, tag="ga8")
            nc.vector.tensor_scalar_mul(out=ga8, in0=t08, scalar1=k1b)
            al8 = batchp.tile([P, BG], F32, tag="al8")
            nc.vector.tensor_tensor(out=al8, in0=ga8, in1=ga8, op=ALU.mult)
            m18 = batchp.tile([P, BG], F32, tag="m18")
            nc.vector.tensor_tensor(out=m18, in0=al8, in1=scc8, op=ALU.mult)
            v18 = batchp.tile([P, BG], F32, tag="v18")
            Ve8 = batchp.tile([P, BG], F32, tag="Ve8")
            if general_g:
                al28 = batchp.tile([P, BG], F32, tag="al28")
                nc.vector.tensor_tensor(out=al28, in0=al8, in1=al8,
                                        op=ALU.mult)
                nc.vector.tensor_tensor(out=v18, in0=al28, in1=q2g8,
                                        op=ALU.mult)
                nc.vector.tensor_scalar(out=Ve8, in0=v18, scalar1=1.0 / I,
                                        scalar2=EPS, op0=ALU.mult,
                                        op1=ALU.add)
            else:
                m28 = batchp.tile([P, BG], F32, tag="m28")
                nc.vector.tensor_tensor(out=m28, in0=m18, in1=m18,
                                        op=ALU.mult)
                nc.vector.tensor_tensor(out=v18, in0=m28, in1=q28,
                                        op=ALU.mult)
                nc.vector.tensor_scalar(out=Ve8, in0=v18, scalar1=KV,
                                        scalar2=EPS, op0=ALU.mult,
                                        op1=ALU.add)
            sq8 = batchp.tile([P, BG], F32, tag="sq8")
            nc.scalar.activation(out=sq8, in_=Ve8, func=AF.Sqrt)
            cr8 = batchp.tile([P, BG], F32, tag="cr8")
            nc.vector.reciprocal(out=cr8, in_=sq8)
            h18 = batchp.tile([P, BG], F32, tag="h18")
            nc.vector.tensor_tensor(out=h18, in0=cr8, in1=cr8, op=ALU.mult)
            h28 = batchp.tile([P, BG], F32, tag="h28")
            nc.vector.tensor_tensor(out=h28, in0=h18, in1=Ve8, op=ALU.mult)
            h38 = batchp.tile([P, BG], F32, tag="h38")
            nc.vector.tensor_scalar(out=h38, in0=h28, scalar1=-0.5,
                                    scalar2=1.5, op0=ALU.mult, op1=ALU.add)
            c8 = batchp.tile([P, BG], F32, tag="c8")
            nc.vector.tensor_tensor(out=c8, in0=cr8, in1=h38, op=ALU.mult)
            if general_g:
                m1g8 = m18
            else:
                m1g8 = batchp.tile([P, BG], F32, tag="m1g8")
                nc.vector.tensor_scalar_mul(out=m1g8, in0=m18, scalar1=g0a)
            mu8 = batchp.tile([P, BG], F32, tag="mu8")
            nc.vector.tensor_tensor(out=mu8, in0=c8, in1=m1g8, op=ALU.mult)
            b8 = batchp.tile([P, BG], F32, tag="b8")
            nc.vector.tensor_scalar(out=b8, in0=mu8, scalar1=1e-5,
                                    scalar2=wdk, op0=ALU.max, op1=ALU.mult)

            # --- scale + store ---
            for j in range(BG):
                r0 = (ib + j) * P
                o2 = o2p.tile([P, H], F32, tag="o2")
                nc.vector.tensor_scalar_mul(out=o2, in0=o_tiles[j],
                                            scalar1=b8[:, j:j + 1])
                nc.sync.dma_start(out=out_d[r0:r0 + P, :], in_=o2)

        # software-pipelined emission: batch ib+1's loads are issued before
        # batch ib's compute so DMA/absmax overlap the previous batch
        state = {}
        phase_a(0, state)
        for ib in range(0, NT, BG):
            if ib + BG < NT:
                phase_a(ib + BG, state)
            phase_bc(ib, state)

    _split_sync_waits(nc)
    return nc


_NC_CACHE = {}


def kernel(x, w_up, w_down, g):
    global LAST_RESULT
    x = np.ascontiguousarray(x, dtype=np.float32)
    w_up = np.ascontiguousarray(w_up, dtype=np.float32)
    w_down = np.ascontiguousarray(w_down, dtype=np.float32)
    g = np.ascontiguousarray(g, dtype=np.float32)

    if abs(float(g[0])) < 1e-30 and np.all(g == g[0]):
        return np.zeros_like(x)

    general = not bool(np.all(g == g[0]))
    key = ("gen" if general else "const")
    if key not in _NC_CACHE:
        _NC_CACHE[key] = build_nc(general)
    nc = _NC_CACHE[key]

    xt = x.reshape(TOK, H)
    wupT = np.ascontiguousarray(w_up.T)    # [H, I]
    wdnT = np.ascontiguousarray(w_down.T)  # [I, H]
    in_maps = [
        {"x": xt[c * TPC:(c + 1) * TPC], "wupT": wupT, "wdnT": wdnT, "g": g}
        for c in range(N_CORES)
    ]
    res = run_bass_kernel_spmd(
        nc, in_maps, list(range(N_CORES)),
        trace=bool(os.environ.get("BASS_TRACE")),
    )
    LAST_RESULT = res
    out = np.concatenate([res.results[c]["out"] for c in range(N_CORES)],
                         axis=0)
    return out.reshape(B, S, H)

